# revision 13
# baseline (speedup 1.0000x reference)
"""AttnAdaIN Trainium2 kernel.

Computation (per batch b):
    F = f_w @ CK + f_b ; G = g_w @ SK + g_b ; Hh = h_w @ STY + h_b   (1x1 convs)
    S = softmax_k(F^T G)          [HW, HW]
    mean = S @ Hh^T ; second = S @ (Hh^T)^2
    std = sqrt(relu(second - mean^2))
    out = std * mvn(content) + mean      (mvn: per-channel mean/var norm, ddof=1)

Kernel strategy (8 NeuronCores, SPMD):
    core i -> (batch b = i//2, query-half h = i%2): 2048 query pixels x 4096 keys.
    Scores: S_pre = CK^T (W^T' SK) with W^T' = g_w^T f_w precomputed host-side,
    so no transposes are needed on-chip. Flash loop with score tiles in
    [k_part=128, q_free=256] orientation; PV matmuls use V-chunks as the
    stationary operand producing accumulators directly in [c, q] orientation
    (the output layout). Softmax runs without max-subtraction (scores are
    O(+-30): exp stays in fp32 range; any global shift cancels in the ratio).
    Denominator accumulated by a ones-vector matmul; 1/d and sqrt are computed
    on ScalarE with a single activation table set via exp/ln.

End-to-end wall time is dominated by the axon tunnel (~70 MB/s in, ~42 MB/s
out), so all wire tensors are fp16 (the PE truncates fp32r operands to 11
mantissa bits anyway, so fp16 inputs cost almost no extra error), content
statistics (mean/rstd over all HW pixels) are computed host-side so only each
core's query-half of content is shipped, and the PJRT executable + device
-resident zero buffers are cached so repeat calls skip tracing/lowering.
"""

import sys
import time

for _p in ("/opt/trn_rl_repo", "/opt/trn_rl_repo/concourse"):
    if _p not in sys.path:
        sys.path.insert(0, _p)

import contextlib
from concurrent.futures import ThreadPoolExecutor

import numpy as np

import concourse.bacc as bacc
import concourse.mybir as mybir
import concourse.tile as tile

F32 = mybir.dt.float32
F32R = mybir.dt.float32r
F16 = mybir.dt.float16
AF = mybir.ActivationFunctionType
ALU = mybir.AluOpType

B, C, H, W = 4, 512, 64, 64
HW = H * W
Q = HW // 2
N_CORES = 8


def build_program(C=512, HW=4096, Q=2048, q_tile=256, with_score_bias=False,
                  with_v_bias=False, n_cores=8):
    """Build + compile the per-core Bass program."""
    assert C % 128 == 0 and HW % 512 == 0 and Q % q_tile == 0
    CC = C // 128          # channel chunks
    NK = HW // 128         # key tiles (flash loop)
    NKS = HW // 512        # 512-wide key slices (G'' precompute)
    NQ = Q // q_tile       # query tiles
    NB = (CC + 1) // 2     # psum accumulator banks per moment (2 c-chunks/bank)
    assert (CC % 2 == 0 and 2 * q_tile <= 512) or CC == 1
    assert 2 * NB + 3 <= 8, "PSUM budget exceeded"

    nc = bacc.Bacc("TRN2", target_bir_lowering=False, debug=False,
                   num_devices=n_cores)

    ck = nc.dram_tensor("ck", [C, Q], F16, kind="ExternalInput")
    sk = nc.dram_tensor("sk", [C, HW], F16, kind="ExternalInput")
    sty = nc.dram_tensor("sty", [C, HW], F16, kind="ExternalInput")
    # content, only this core's Q query columns (channel stats arrive
    # precomputed host-side as musr)
    ct = nc.dram_tensor("ct", [C, Q], F16, kind="ExternalInput")
    wT = nc.dram_tensor("wT", [C, C], F16, kind="ExternalInput")
    hwT = nc.dram_tensor("hwT", [C, C], F16, kind="ExternalInput")
    # musr[:, :CC] = per-channel mean, musr[:, CC:] = per-channel 1/std
    musr_d = nc.dram_tensor("musr", [128, 2 * (C // 128)], F32,
                            kind="ExternalInput")
    onesk_d = nc.dram_tensor("onesk", [128, 1], F32R, kind="ExternalInput")
    if with_score_bias or with_v_bias:
        onesr_d = nc.dram_tensor("onesr", [1, 128], F16,
                                 kind="ExternalInput")
    if with_score_bias:
        rbias = nc.dram_tensor("rbias", [1, HW], F16, kind="ExternalInput")
    if with_v_bias:
        hb = nc.dram_tensor("hb", [1, C], F16, kind="ExternalInput")
    out = nc.dram_tensor("out", [C, Q], F16, kind="ExternalOutput")

    ckr = ck.rearrange("(c p) q -> c p q", p=128)    # [CC, 128, Q]
    skr = sk.rearrange("(c p) k -> c p k", p=128)
    styr = sty.rearrange("(c p) k -> c p k", p=128)
    ctr = ct.rearrange("(c p) k -> c p k", p=128)
    wTr = wT.rearrange("(c p) a -> c p a", p=128)
    hwTr = hwT.rearrange("(c p) a -> c p a", p=128)
    outr = out.rearrange("(c p) q -> c p q", p=128)

    with tile.TileContext(nc) as tc, contextlib.ExitStack() as ctx:
        persist = ctx.enter_context(tc.tile_pool(name="persist", bufs=1))
        ckpool = ctx.enter_context(tc.tile_pool(name="ckpool", bufs=2))
        ppool = ctx.enter_context(tc.tile_pool(name="ppool", bufs=4))
        v2pool = ctx.enter_context(tc.tile_pool(name="v2pool", bufs=4))
        epool = ctx.enter_context(tc.tile_pool(name="epool", bufs=2))
        opool = ctx.enter_context(tc.tile_pool(name="opool", bufs=2))
        ps_st = ctx.enter_context(
            tc.tile_pool(name="ps_st", bufs=3, space="PSUM"))
        ps_acc = ctx.enter_context(
            tc.tile_pool(name="ps_acc", bufs=1, space="PSUM"))
        ps_d = ctx.enter_context(
            tc.tile_pool(name="ps_d", bufs=1, space="PSUM"))
        dpool = ctx.enter_context(
            tc.tile_pool(name="dpool", bufs=2, space="DRAM"))

        # ---- constants ----
        ones_k = persist.tile([128, 1], F32R, tag="ones_k")
        nc.sync.dma_start(out=ones_k, in_=onesk_d[:])
        if with_score_bias or with_v_bias:
            ones_r = persist.tile([1, 128], F16, tag="ones_r")
            nc.sync.dma_start(out=ones_r, in_=onesr_d[:])
        shift_sb = persist.tile([128, 1], F32, tag="shift")
        nc.vector.memset(shift_sb, -30.0)

        g2 = persist.tile([128, CC, HW], F16, tag="g2")
        vsb = persist.tile([128, NK, C], F32R, tag="v")
        musr = persist.tile([128, 2 * CC], F32, tag="musr")
        nc.sync.dma_start(out=musr, in_=musr_d[:])
        if with_score_bias:
            r_sb = persist.tile([1, HW], F16, tag="rbias")
            nc.sync.dma_start(out=r_sb, in_=rbias[:])
        if with_v_bias:
            hb_sb = persist.tile([1, C], F16, tag="hb")
            nc.sync.dma_start(out=hb_sb, in_=hb[:])

        # ---- phase 0: weights, G'' and V precompute ----
        with tc.tile_pool(name="ph0", bufs=1) as ph0, \
             tc.tile_pool(name="ph0s", bufs=2) as ph0s:
            wT_sb = ph0.tile([128, CC, C], F16, tag="wT")
            hwT_sb = ph0.tile([128, CC, C], F16, tag="hwT")
            for c in range(CC):
                nc.sync.dma_start(out=wT_sb[:, c, :], in_=wTr[c])
                nc.sync.dma_start(out=hwT_sb[:, c, :], in_=hwTr[c])

            # G'' = W^T' SK  (score stationary operand), layout [c, k]
            for ks in range(2 * NKS):
                sl = slice(ks * 256, (ks + 1) * 256)
                sks = ph0s.tile([128, CC, 256], F16, tag="sk_stream")
                for b in range(CC):
                    nc.sync.dma_start(out=sks[:, b, :], in_=skr[b][:, sl])
                for a in range(CC):
                    gps = ps_st.tile([128, 256], F32, tag="st", name="gps")
                    for b in range(CC):
                        nc.tensor.matmul(
                            gps,
                            lhsT=wT_sb[:, b, a * 128:(a + 1) * 128],
                            rhs=sks[:, b, :],
                            start=(b == 0), stop=(b == CC - 1))
                    nc.scalar.copy(out=g2[:, a, sl], in_=gps)

            # V = STY^T hwT  ([k, c] in 128-row blocks)
            for kt in range(NK):
                sl = slice(kt * 128, (kt + 1) * 128)
                sts = ph0s.tile([128, CC, 128], F16, tag="sty_stream")
                for b in range(CC):
                    nc.sync.dma_start(out=sts[:, b, :], in_=styr[b][:, sl])
                vps = ps_st.tile([128, 512], F32, tag="st")
                for b in range(CC):
                    nc.tensor.matmul(vps[:, :C],
                                     lhsT=sts[:, b, :],
                                     rhs=hwT_sb[:, b, :],
                                     start=(b == 0), stop=(b == CC - 1))
                if with_v_bias:
                    nc.tensor.matmul(vps[:, :C],
                                     lhsT=ones_r,
                                     rhs=hb_sb,
                                     start=False, stop=True,
                                     skip_group_check=True)
                nc.scalar.copy(out=vsb[:, kt, :], in_=vps[:, :C])

        # ---- flash main loop ----
        for qt in range(NQ):
            qsl = slice(qt * q_tile, (qt + 1) * q_tile)
            ckq = ckpool.tile([128, CC, q_tile], F16, tag="ckq")
            for c in range(CC):
                nc.sync.dma_start(out=ckq[:, c, :], in_=ckr[c][:, qsl])

            acc1 = [ps_acc.tile([128, 512], F32, tag=f"acc1_{i}",
                                name=f"acc1_{i}") for i in range(NB)]
            acc2 = [ps_acc.tile([128, 512], F32, tag=f"acc2_{i}",
                                name=f"acc2_{i}") for i in range(NB)]
            dps = ps_d.tile([1, q_tile], F32, tag="d")

            def acc_ap(accs, c):
                return accs[c // 2][:, (c % 2) * q_tile:(c % 2 + 1) * q_tile]

            # NOTE: start=True clears has_written bits for the WHOLE psum
            # bank, so each bank (2 c-chunks) forms a single accumulation
            # group: only its first matmul sets start.
            def emit_pv(kt, p, v2):
                nc.tensor.matmul(dps, lhsT=ones_k, rhs=p,
                                 start=(kt == 0), stop=(kt == NK - 1),
                                 skip_group_check=True)
                for acc, lhs in ((acc1, vsb[:, kt, :]), (acc2, v2)):
                    for c in range(CC):
                        csl = slice(c * 128, (c + 1) * 128)
                        nc.tensor.matmul(acc_ap(acc, c),
                                         lhsT=lhs[:, csl],
                                         rhs=p,
                                         start=(kt == 0 and c % 2 == 0),
                                         stop=(kt == NK - 1 and
                                               (c % 2 == 1 or c == CC - 1)),
                                         skip_group_check=True)

            # software pipeline: QK(kt) is emitted before PV(kt-1) so the PE
            # has score matmuls to run while ScalarE computes exp(kt-1).
            pending = []
            for kt in range(NK):
                ksl = slice(kt * 128, (kt + 1) * 128)
                st = ps_st.tile([128, q_tile], F32, tag="st")
                for c in range(CC):
                    nc.tensor.matmul(st,
                                     lhsT=g2[:, c, ksl],
                                     rhs=ckq[:, c, :],
                                     start=(c == 0),
                                     stop=(c == CC - 1 and not with_score_bias))
                if with_score_bias:
                    nc.tensor.matmul(st, lhsT=r_sb[:, ksl],
                                     rhs=ones_r[:, :q_tile],
                                     start=False, stop=True,
                                     skip_group_check=True)
                p = ppool.tile([128, q_tile], F32R, tag="p")
                nc.scalar.activation(out=p, in_=st, func=AF.Exp, bias=shift_sb)
                v2 = v2pool.tile([128, C], F32R, tag="v2")
                nc.gpsimd.tensor_mul(v2, vsb[:, kt, :], vsb[:, kt, :])
                pending.append((kt, p, v2))
                if len(pending) > 2:
                    emit_pv(*pending.pop(0))
            for item in pending:
                emit_pv(*item)

            # ---- epilogue for this q_tile ----
            rd = epool.tile([1, q_tile], F32, tag="rd", bufs=1)
            nc.vector.reciprocal(out=rd, in_=dps)
            rd_dram = dpool.tile([1, q_tile], F32, tag="rd_dram")
            nc.sync.dma_start(out=rd_dram, in_=rd)
            rdb = epool.tile([128, q_tile], F32, tag="rdb", bufs=1)
            nc.sync.dma_start(out=rdb,
                              in_=rd_dram.to_broadcast([128, q_tile]))

            avs, a2s = [], []
            for c in range(CC):
                av = epool.tile([128, q_tile], F32, tag=f"av{c}", name=f"av{c}", bufs=1)
                nc.scalar.copy(out=av, in_=acc_ap(acc1, c))
                a2 = epool.tile([128, q_tile], F32, tag=f"a2{c}", name=f"a2{c}", bufs=1)
                nc.scalar.copy(out=a2, in_=acc_ap(acc2, c))
                avs.append(av)
                a2s.append(a2)

            for c in range(CC):
                ctq = epool.tile([128, q_tile], F16, tag="ctq")
                nc.sync.dma_start(out=ctq, in_=ctr[c][:, qsl])
                mean = avs[c]
                nc.vector.tensor_mul(mean, avs[c], rdb)
                e2 = a2s[c]
                nc.vector.tensor_mul(e2, a2s[c], rdb)
                var = epool.tile([128, q_tile], F32, tag="var", bufs=1)
                nc.vector.tensor_mul(var, mean, mean)
                nc.vector.scalar_tensor_tensor(
                    out=var, in0=var, scalar=-1.0, in1=e2,
                    op0=ALU.mult, op1=ALU.add)
                nc.vector.tensor_scalar_max(var, var, 1e-38)
                std = var
                nc.scalar.activation(out=std, in_=var, func=AF.Ln)
                nc.scalar.activation(out=std, in_=std, func=AF.Exp, scale=0.5)
                normc = epool.tile([128, q_tile], F32, tag="normc", bufs=1)
                nc.vector.tensor_scalar(
                    out=normc, in0=ctq,
                    scalar1=musr[:, c:c + 1], scalar2=musr[:, CC + c:CC + c + 1],
                    op0=ALU.subtract, op1=ALU.mult)
                o = opool.tile([128, q_tile], F16, tag="o")
                nc.vector.tensor_mul(std, std, normc)
                nc.vector.tensor_add(o, std, mean)
                nc.sync.dma_start(out=outr[c][:, qsl], in_=o)

    # Force exp/ln/copy onto the shared natural_log_exp_and_others table
    # set: the default per-function choice alternates exp_and_others <->
    # natural_log, costing ~2.7us per ACT_TABLE_LOAD, dozens of times.
    import concourse.bacc as bacc_mod
    _orig_tables = bacc_mod.get_activation_tables
    _keep = "natural_log_exp_and_others"
    _strip = {AF.Exp, AF.Ln, AF.Copy, AF.Identity}

    def _patched_tables(arch):
        t = _orig_tables(arch)
        for name, fns in t.items():
            if name != _keep:
                t[name] = fns - _strip
        return t

    bacc_mod.get_activation_tables = _patched_tables
    try:
        nc.compile()
    finally:
        bacc_mod.get_activation_tables = _orig_tables
    return nc


class _Exec:
    """Compiled program + cached PJRT executable + reusable buffers."""

    def __init__(self, key):
        import jax
        from jax.sharding import Mesh, NamedSharding, PartitionSpec
        from jax.experimental.shard_map import shard_map
        import concourse.bass2jax as bass2jax

        with_r, with_hb = key
        self.nc = nc = build_program(with_score_bias=with_r,
                                     with_v_bias=with_hb)
        bass2jax.install_neuronx_cc_hook()

        partition_name = (
            nc.partition_id_tensor.name if nc.partition_id_tensor else None)
        in_names, out_names, out_avals, zero_outs = [], [], [], []
        for alloc in nc.m.functions[0].allocations:
            if not isinstance(alloc, mybir.MemoryLocationSet):
                continue
            name = alloc.memorylocations[0].name
            if alloc.kind == "ExternalInput":
                if name != partition_name:
                    in_names.append(name)
            elif alloc.kind == "ExternalOutput":
                shape = tuple(alloc.tensor_shape)
                dtype = mybir.dt.np(alloc.dtype)
                out_names.append(name)
                out_avals.append(jax.core.ShapedArray(shape, dtype))
                zero_outs.append(np.zeros((N_CORES * shape[0], *shape[1:]),
                                          dtype))
        self.in_names = in_names
        self.out_names = out_names
        n_ops = len(in_names) + len(out_names)

        def _body(*args):
            operands = list(args)
            if partition_name is not None:
                operands.append(bass2jax.partition_id_tensor())
            outs = bass2jax._bass_exec_p.bind(
                *operands,
                out_avals=tuple(out_avals),
                in_names=tuple(in_names + out_names +
                               ([partition_name] if partition_name else [])),
                out_names=tuple(out_names),
                lowering_input_output_aliases=(),
                sim_require_finite=True,
                sim_require_nnan=True,
                nc=nc,
            )
            return tuple(outs)

        devices = jax.devices()[:N_CORES]
        mesh = Mesh(np.asarray(devices), ("core",))
        self.sharding = NamedSharding(mesh, PartitionSpec("core"))
        self.fn = jax.jit(
            shard_map(_body, mesh=mesh,
                      in_specs=(PartitionSpec("core"),) * n_ops,
                      out_specs=(PartitionSpec("core"),) * len(out_names),
                      check_rep=False),
            keep_unused=True,
        )
        self.dev_zeros = [jax.device_put(z, self.sharding) for z in zero_outs]
        jax.block_until_ready(self.dev_zeros)
        # reusable host-side concat buffers, keyed by input name
        self.host_buf = {}

    def buf(self, name, shape, dtype):
        b = self.host_buf.get(name)
        if b is None or b.shape != shape or b.dtype != dtype:
            b = np.empty(shape, dtype)
            self.host_buf[name] = b
        return b

    def run(self, arrays):
        """arrays: dict name -> concat ndarray [N_CORES*rows, cols]."""
        import jax
        dev_in = [jax.device_put(arrays[n], self.sharding)
                  for n in self.in_names]
        outs = self.fn(*dev_in, *self.dev_zeros)
        return {n: np.asarray(o) for n, o in zip(self.out_names, outs)}


_EXEC_CACHE = {}


def _get_exec(key):
    if key not in _EXEC_CACHE:
        _EXEC_CACHE[key] = _Exec(key)
    return _EXEC_CACHE[key]


def prepare_inputs(ex, content, style, content_key, style_key, f_w, f_b,
                   g_w, g_b, h_w, h_b):
    """Fill ex's concat host buffers (fp16 wire format). Returns dict."""
    content = np.asarray(content)
    style = np.asarray(style)
    content_key = np.asarray(content_key)
    style_key = np.asarray(style_key)
    CC = C // 128

    wT = ex.buf("wT", (N_CORES * C, C), np.float16)
    hwT = ex.buf("hwT", (N_CORES * C, C), np.float16)
    wT_1 = (np.asarray(g_w).T.astype(np.float32) @
            np.asarray(f_w).astype(np.float32)).astype(np.float16)
    hwT_1 = np.asarray(h_w).T.astype(np.float16)
    for core in range(N_CORES):
        wT[core * C:(core + 1) * C] = wT_1
        hwT[core * C:(core + 1) * C] = hwT_1

    # host-side per-(b, channel) stats over all HW pixels (ddof=1 + EPS)
    cf = content.reshape(B, C, HW)
    mu_b = cf.mean(axis=2)                                   # [B, C]
    var_b = cf.var(axis=2, ddof=1) + 1e-5
    rstd_b = 1.0 / np.sqrt(var_b)
    musr = ex.buf("musr", (N_CORES * 128, 2 * CC), np.float32)
    for core in range(N_CORES):
        b = core // 2
        blk = musr[core * 128:(core + 1) * 128]
        blk[:, :CC] = mu_b[b].reshape(CC, 128).T
        blk[:, CC:] = rstd_b[b].reshape(CC, 128).T

    ckb = ex.buf("ck", (N_CORES * C, Q), np.float16)
    skb = ex.buf("sk", (N_CORES * C, HW), np.float16)
    styb = ex.buf("sty", (N_CORES * C, HW), np.float16)
    ctb = ex.buf("ct", (N_CORES * C, Q), np.float16)

    def fill(core):
        b, h = divmod(core, 2)
        r = slice(core * C, (core + 1) * C)
        qs = slice(h * Q, (h + 1) * Q)
        ckb[r] = content_key[b].reshape(C, HW)[:, qs]
        ctb[r] = content[b].reshape(C, HW)[:, qs]
        skb[r] = style_key[b].reshape(C, HW)
        styb[r] = style[b].reshape(C, HW)

    with ThreadPoolExecutor(max_workers=8) as tp:
        list(tp.map(fill, range(N_CORES)))

    onesk = ex.buf("onesk", (N_CORES * 128, 1), np.float32)
    onesk[:] = 1.0
    m = {"ck": ckb, "sk": skb, "sty": styb, "ct": ctb,
         "wT": wT, "hwT": hwT, "musr": musr, "onesk": onesk}

    with_r = bool(np.any(f_b))
    with_hb = bool(np.any(h_b))
    if with_r or with_hb:
        onesr = ex.buf("onesr", (N_CORES * 1, 128), np.float16)
        onesr[:] = 1.0
        m["onesr"] = onesr
    if with_r:
        u = np.asarray(g_w).T.astype(np.float64) @ np.asarray(f_b, np.float64)
        rb = ex.buf("rbias", (N_CORES * 1, HW), np.float16)
        for core in range(N_CORES):
            b = core // 2
            rb[core] = (u @ style_key[b].reshape(C, HW).astype(np.float64))
        m["rbias"] = rb
    if with_hb:
        hb = ex.buf("hb", (N_CORES * 1, C), np.float16)
        hb[:] = np.asarray(h_b, np.float16)[None, :]
        m["hb"] = hb
    return m


def _variant_key(f_b, h_b):
    return (bool(np.any(f_b)), bool(np.any(h_b)))


def kernel(**inputs):
    key = _variant_key(inputs["f_b"], inputs["h_b"])
    ex = _get_exec(key)
    arrays = prepare_inputs(ex, **inputs)
    res = ex.run(arrays)
    o = res["out"]                               # [8*C, Q] fp16
    out = np.empty((B, C, HW), np.float32)
    for core in range(N_CORES):
        b, h = divmod(core, 2)
        out[b][:, h * Q:(h + 1) * Q] = o[core * C:(core + 1) * C]
    return out.reshape(B, C, H, W)


if __name__ == "__main__":
    rng = np.random.default_rng(0)
    inputs = {
        "content": rng.standard_normal((B, C, H, W)).astype(np.float32),
        "style": rng.standard_normal((B, C, H, W)).astype(np.float32),
        "content_key": rng.standard_normal((B, C, H, W)).astype(np.float32),
        "style_key": rng.standard_normal((B, C, H, W)).astype(np.float32),
        "f_w": (rng.standard_normal((C, C)) * 0.02).astype(np.float32),
        "f_b": np.zeros(C, np.float32),
        "g_w": (rng.standard_normal((C, C)) * 0.02).astype(np.float32),
        "g_b": np.zeros(C, np.float32),
        "h_w": (rng.standard_normal((C, C)) * 0.02).astype(np.float32),
        "h_b": np.zeros(C, np.float32),
    }
    t0 = time.time()
    out = kernel(**inputs)
    print("kernel done", out.shape, out.dtype, time.time() - t0)
    t0 = time.time()
    out = kernel(**inputs)
    print("second call", time.time() - t0)


# revision 22
# speedup vs baseline: 1.2164x; 1.2164x over previous
"""AttnAdaIN Trainium2 kernel.

Computation (per batch b):
    F = f_w @ CK + f_b ; G = g_w @ SK + g_b ; Hh = h_w @ STY + h_b   (1x1 convs)
    S = softmax_k(F^T G)          [HW, HW]
    mean = S @ Hh^T ; second = S @ (Hh^T)^2
    std = sqrt(relu(second - mean^2))
    out = std * mvn(content) + mean      (mvn: per-channel mean/var norm, ddof=1)

Kernel strategy (8 NeuronCores, SPMD):
    core i -> (batch b = i//2, query-half h = i%2): 2048 query pixels x 4096 keys.
    Scores: S_pre = CK^T (W^T' SK) with W^T' = g_w^T f_w precomputed host-side,
    so no transposes are needed on-chip. Flash loop with score tiles in
    [k_part=128, q_free=256] orientation; PV matmuls use V-chunks as the
    stationary operand producing accumulators directly in [c, q] orientation
    (the output layout). Softmax runs without max-subtraction (scores are
    O(+-30): exp stays in fp32 range; any global shift cancels in the ratio).
    Denominator accumulated by a ones-vector matmul; 1/d and sqrt are computed
    on ScalarE with a single activation table set via exp/ln.

End-to-end wall time is dominated by the axon tunnel (~70 MB/s in, ~42 MB/s
out), so all wire tensors are fp16 (the PE truncates fp32r operands to 11
mantissa bits anyway, so fp16 inputs cost almost no extra error), content
statistics (mean/rstd over all HW pixels) are computed host-side so only each
core's query-half of content is shipped, and the PJRT executable + device
-resident zero buffers are cached so repeat calls skip tracing/lowering.
"""

import sys
import time

for _p in ("/opt/trn_rl_repo", "/opt/trn_rl_repo/concourse"):
    if _p not in sys.path:
        sys.path.insert(0, _p)

import contextlib
from concurrent.futures import ThreadPoolExecutor

import numpy as np

import concourse.bacc as bacc
import concourse.mybir as mybir
import concourse.tile as tile

F32 = mybir.dt.float32
F32R = mybir.dt.float32r
F16 = mybir.dt.float16
AF = mybir.ActivationFunctionType
ALU = mybir.AluOpType

B, C, H, W = 4, 512, 64, 64
HW = H * W
Q = HW // 2
N_CORES = 8


def build_program(C=512, HW=4096, Q=2048, q_tile=256, with_score_bias=False,
                  with_v_bias=False, n_cores=8):
    """Build + compile the per-core Bass program."""
    assert C % 128 == 0 and HW % 512 == 0 and Q % q_tile == 0
    CC = C // 128          # channel chunks
    NK = HW // 128         # key tiles (flash loop)
    NKS = HW // 512        # 512-wide key slices (G'' precompute)
    NQ = Q // q_tile       # query tiles
    NB = (CC + 1) // 2     # psum accumulator banks per moment (2 c-chunks/bank)
    assert (CC % 2 == 0 and 2 * q_tile <= 512) or CC == 1
    assert 2 * NB + 3 <= 8, "PSUM budget exceeded"

    nc = bacc.Bacc("TRN2", target_bir_lowering=False, debug=False,
                   num_devices=n_cores)

    # catq = [CK | CT]: this core's Q query columns of content_key and
    # content (channel stats arrive precomputed host-side as musr).
    catq = nc.dram_tensor("catq", [C, 2 * Q], F16, kind="ExternalInput")
    # gath = [SK | STY | wT | hwT] rows h*C/2..(h+1)*C/2 (this core's
    # channel-half of its batch's shared tensors). The pair's halves are
    # AllGathered on-chip over NeuronLink so each tensor crosses the host
    # tunnel exactly once.
    GW = 2 * HW + 2 * C
    gath = nc.dram_tensor("gath", [C // 2, GW], F16, kind="ExternalInput")
    # musr[:, :CC] = per-channel mean, musr[:, CC:] = per-channel 1/std
    musr_d = nc.dram_tensor("musr", [128, 2 * (C // 128)], F32,
                            kind="ExternalInput")
    onesk_d = nc.dram_tensor("onesk", [128, 1], F32R, kind="ExternalInput")
    if with_score_bias or with_v_bias:
        onesr_d = nc.dram_tensor("onesr", [1, 128], F16,
                                 kind="ExternalInput")
    if with_score_bias:
        rbias = nc.dram_tensor("rbias", [1, HW], F16, kind="ExternalInput")
    if with_v_bias:
        hb = nc.dram_tensor("hb", [1, C], F16, kind="ExternalInput")
    out = nc.dram_tensor("out", [C, Q], F16, kind="ExternalOutput")

    catqr = catq.rearrange("(c p) q -> c p q", p=128)  # [CC, 128, 2Q]
    outr = out.rearrange("(c p) q -> c p q", p=128)

    with tile.TileContext(nc) as tc, contextlib.ExitStack() as ctx:
        persist = ctx.enter_context(tc.tile_pool(name="persist", bufs=1))
        ckpool = ctx.enter_context(tc.tile_pool(name="ckpool", bufs=2))
        ppool = ctx.enter_context(tc.tile_pool(name="ppool", bufs=4))
        v2pool = ctx.enter_context(tc.tile_pool(name="v2pool", bufs=4))
        epool = ctx.enter_context(tc.tile_pool(name="epool", bufs=2))
        opool = ctx.enter_context(tc.tile_pool(name="opool", bufs=2))
        ps_st = ctx.enter_context(
            tc.tile_pool(name="ps_st", bufs=3, space="PSUM"))
        ps_acc = ctx.enter_context(
            tc.tile_pool(name="ps_acc", bufs=1, space="PSUM"))
        ps_d = ctx.enter_context(
            tc.tile_pool(name="ps_d", bufs=1, space="PSUM"))
        dpool = ctx.enter_context(
            tc.tile_pool(name="dpool", bufs=2, space="DRAM"))
        ccpool = ctx.enter_context(
            tc.tile_pool(name="ccpool", bufs=1, space="DRAM"))

        # ---- phase -1: AllGather the pair-shared tensors ----
        # stage ExternalInput -> internal DRAM (collectives can't read IO),
        # then pairwise AllGather: gathered rows 0..C/2 come from the even
        # core (channels [0, C/2)), rows C/2..C from the odd core.
        gsrc = ccpool.tile([C // 2, GW], F16, tag="gsrc")
        gall = ccpool.tile([C, GW], F16, tag="gall")
        with tc.tile_pool(name="stg", bufs=2) as stg:
            gathr = gath.rearrange("(c p) f -> c p f", p=128)
            gsrcr = gsrc.rearrange("(c p) f -> c p f", p=128)
            for c in range(C // 256):
                s = stg.tile([128, GW], F16, tag="stage")
                nc.sync.dma_start(out=s, in_=gathr[c])
                nc.sync.dma_start(out=gsrcr[c], in_=s)
        nc.gpsimd.collective_compute(
            "AllGather", mybir.AluOpType.bypass,
            replica_groups=[[2 * i, 2 * i + 1] for i in range(n_cores // 2)],
            ins=[gsrc[:]], outs=[gall[:]],
        )
        gallr = gall.rearrange("(c p) f -> c p f", p=128)  # [CC, 128, GW]

        def skr(c, sl):
            return gallr[c][:, sl.start:sl.stop]

        def styr(c, sl):
            return gallr[c][:, HW + sl.start:HW + sl.stop]

        def wTr(c):
            return gallr[c][:, 2 * HW:2 * HW + C]

        def hwTr(c):
            return gallr[c][:, 2 * HW + C:2 * HW + 2 * C]

        def ckr(c, sl):
            return catqr[c][:, sl.start:sl.stop]

        def ctr(c, sl):
            return catqr[c][:, Q + sl.start:Q + sl.stop]

        # ---- constants ----
        ones_k = persist.tile([128, 1], F32R, tag="ones_k")
        nc.sync.dma_start(out=ones_k, in_=onesk_d[:])
        if with_score_bias or with_v_bias:
            ones_r = persist.tile([1, 128], F16, tag="ones_r")
            nc.sync.dma_start(out=ones_r, in_=onesr_d[:])
        shift_sb = persist.tile([128, 1], F32, tag="shift")
        nc.vector.memset(shift_sb, -30.0)

        g2 = persist.tile([128, CC, HW], F16, tag="g2")
        vsb = persist.tile([128, NK, C], F32R, tag="v")
        musr = persist.tile([128, 2 * CC], F32, tag="musr")
        nc.sync.dma_start(out=musr, in_=musr_d[:])
        if with_score_bias:
            r_sb = persist.tile([1, HW], F16, tag="rbias")
            nc.sync.dma_start(out=r_sb, in_=rbias[:])
        if with_v_bias:
            hb_sb = persist.tile([1, C], F16, tag="hb")
            nc.sync.dma_start(out=hb_sb, in_=hb[:])

        # ---- phase 0: weights, G'' and V precompute ----
        with tc.tile_pool(name="ph0", bufs=1) as ph0, \
             tc.tile_pool(name="ph0s", bufs=2) as ph0s:
            wT_sb = ph0.tile([128, CC, C], F16, tag="wT")
            hwT_sb = ph0.tile([128, CC, C], F16, tag="hwT")
            for c in range(CC):
                nc.sync.dma_start(out=wT_sb[:, c, :], in_=wTr(c))
                nc.sync.dma_start(out=hwT_sb[:, c, :], in_=hwTr(c))

            # G'' = W^T' SK  (score stationary operand), layout [c, k]
            for ks in range(2 * NKS):
                sl = slice(ks * 256, (ks + 1) * 256)
                sks = ph0s.tile([128, CC, 256], F16, tag="sk_stream")
                for b in range(CC):
                    nc.sync.dma_start(out=sks[:, b, :], in_=skr(b, sl))
                for a in range(CC):
                    gps = ps_st.tile([128, 256], F32, tag="st", name="gps")
                    for b in range(CC):
                        nc.tensor.matmul(
                            gps,
                            lhsT=wT_sb[:, b, a * 128:(a + 1) * 128],
                            rhs=sks[:, b, :],
                            start=(b == 0), stop=(b == CC - 1))
                    nc.scalar.copy(out=g2[:, a, sl], in_=gps)

            # V = STY^T hwT  ([k, c] in 128-row blocks)
            for kt in range(NK):
                sl = slice(kt * 128, (kt + 1) * 128)
                sts = ph0s.tile([128, CC, 128], F16, tag="sty_stream")
                for b in range(CC):
                    nc.sync.dma_start(out=sts[:, b, :], in_=styr(b, sl))
                vps = ps_st.tile([128, 512], F32, tag="st")
                for b in range(CC):
                    nc.tensor.matmul(vps[:, :C],
                                     lhsT=sts[:, b, :],
                                     rhs=hwT_sb[:, b, :],
                                     start=(b == 0), stop=(b == CC - 1))
                if with_v_bias:
                    nc.tensor.matmul(vps[:, :C],
                                     lhsT=ones_r,
                                     rhs=hb_sb,
                                     start=False, stop=True,
                                     skip_group_check=True)
                nc.scalar.copy(out=vsb[:, kt, :], in_=vps[:, :C])

        # ---- flash main loop ----
        for qt in range(NQ):
            qsl = slice(qt * q_tile, (qt + 1) * q_tile)
            ckq = ckpool.tile([128, CC, q_tile], F16, tag="ckq")
            for c in range(CC):
                nc.sync.dma_start(out=ckq[:, c, :], in_=ckr(c, qsl))

            acc1 = [ps_acc.tile([128, 512], F32, tag=f"acc1_{i}",
                                name=f"acc1_{i}") for i in range(NB)]
            acc2 = [ps_acc.tile([128, 512], F32, tag=f"acc2_{i}",
                                name=f"acc2_{i}") for i in range(NB)]
            dps = ps_d.tile([1, q_tile], F32, tag="d")

            def acc_ap(accs, c):
                return accs[c // 2][:, (c % 2) * q_tile:(c % 2 + 1) * q_tile]

            # NOTE: start=True clears has_written bits for the WHOLE psum
            # bank, so each bank (2 c-chunks) forms a single accumulation
            # group: only its first matmul sets start.
            def emit_pv(kt, p, v2):
                nc.tensor.matmul(dps, lhsT=ones_k, rhs=p,
                                 start=(kt == 0), stop=(kt == NK - 1),
                                 skip_group_check=True)
                for acc, lhs in ((acc1, vsb[:, kt, :]), (acc2, v2)):
                    for c in range(CC):
                        csl = slice(c * 128, (c + 1) * 128)
                        nc.tensor.matmul(acc_ap(acc, c),
                                         lhsT=lhs[:, csl],
                                         rhs=p,
                                         start=(kt == 0 and c % 2 == 0),
                                         stop=(kt == NK - 1 and
                                               (c % 2 == 1 or c == CC - 1)),
                                         skip_group_check=True)

            # software pipeline: QK(kt) is emitted before PV(kt-1) so the PE
            # has score matmuls to run while ScalarE computes exp(kt-1).
            pending = []
            for kt in range(NK):
                ksl = slice(kt * 128, (kt + 1) * 128)
                st = ps_st.tile([128, q_tile], F32, tag="st")
                for c in range(CC):
                    nc.tensor.matmul(st,
                                     lhsT=g2[:, c, ksl],
                                     rhs=ckq[:, c, :],
                                     start=(c == 0),
                                     stop=(c == CC - 1 and not with_score_bias))
                if with_score_bias:
                    nc.tensor.matmul(st, lhsT=r_sb[:, ksl],
                                     rhs=ones_r[:, :q_tile],
                                     start=False, stop=True,
                                     skip_group_check=True)
                p = ppool.tile([128, q_tile], F32R, tag="p")
                nc.scalar.activation(out=p, in_=st, func=AF.Exp, bias=shift_sb)
                v2 = v2pool.tile([128, C], F32R, tag="v2")
                nc.gpsimd.tensor_mul(v2, vsb[:, kt, :], vsb[:, kt, :])
                pending.append((kt, p, v2))
                if len(pending) > 2:
                    emit_pv(*pending.pop(0))
            for item in pending:
                emit_pv(*item)

            # ---- epilogue for this q_tile ----
            rd = epool.tile([1, q_tile], F32, tag="rd", bufs=1)
            nc.vector.reciprocal(out=rd, in_=dps)
            rd_dram = dpool.tile([1, q_tile], F32, tag="rd_dram")
            nc.sync.dma_start(out=rd_dram, in_=rd)
            rdb = epool.tile([128, q_tile], F32, tag="rdb", bufs=1)
            nc.sync.dma_start(out=rdb,
                              in_=rd_dram.to_broadcast([128, q_tile]))

            avs, a2s = [], []
            for c in range(CC):
                av = epool.tile([128, q_tile], F32, tag=f"av{c}", name=f"av{c}", bufs=1)
                nc.scalar.copy(out=av, in_=acc_ap(acc1, c))
                a2 = epool.tile([128, q_tile], F32, tag=f"a2{c}", name=f"a2{c}", bufs=1)
                nc.scalar.copy(out=a2, in_=acc_ap(acc2, c))
                avs.append(av)
                a2s.append(a2)

            for c in range(CC):
                ctq = epool.tile([128, q_tile], F16, tag="ctq")
                nc.sync.dma_start(out=ctq, in_=ctr(c, qsl))
                mean = avs[c]
                nc.vector.tensor_mul(mean, avs[c], rdb)
                e2 = a2s[c]
                nc.vector.tensor_mul(e2, a2s[c], rdb)
                var = epool.tile([128, q_tile], F32, tag="var", bufs=1)
                nc.vector.tensor_mul(var, mean, mean)
                nc.vector.scalar_tensor_tensor(
                    out=var, in0=var, scalar=-1.0, in1=e2,
                    op0=ALU.mult, op1=ALU.add)
                nc.vector.tensor_scalar_max(var, var, 1e-38)
                std = var
                nc.scalar.activation(out=std, in_=var, func=AF.Ln)
                nc.scalar.activation(out=std, in_=std, func=AF.Exp, scale=0.5)
                normc = epool.tile([128, q_tile], F32, tag="normc", bufs=1)
                nc.vector.tensor_scalar(
                    out=normc, in0=ctq,
                    scalar1=musr[:, c:c + 1], scalar2=musr[:, CC + c:CC + c + 1],
                    op0=ALU.subtract, op1=ALU.mult)
                o = opool.tile([128, q_tile], F16, tag="o")
                nc.vector.tensor_mul(std, std, normc)
                nc.vector.tensor_add(o, std, mean)
                nc.sync.dma_start(out=outr[c][:, qsl], in_=o)

    # Force exp/ln/copy onto the shared natural_log_exp_and_others table
    # set: the default per-function choice alternates exp_and_others <->
    # natural_log, costing ~2.7us per ACT_TABLE_LOAD, dozens of times.
    import concourse.bacc as bacc_mod
    _orig_tables = bacc_mod.get_activation_tables
    _keep = "natural_log_exp_and_others"
    _strip = {AF.Exp, AF.Ln, AF.Copy, AF.Identity}

    def _patched_tables(arch):
        t = _orig_tables(arch)
        for name, fns in t.items():
            if name != _keep:
                t[name] = fns - _strip
        return t

    bacc_mod.get_activation_tables = _patched_tables
    try:
        nc.compile()
    finally:
        bacc_mod.get_activation_tables = _orig_tables
    return nc


class _Exec:
    """Compiled program + cached PJRT executable + reusable buffers."""

    def __init__(self, key):
        import jax
        from jax.sharding import Mesh, NamedSharding, PartitionSpec
        from jax.experimental.shard_map import shard_map
        import concourse.bass2jax as bass2jax

        with_r, with_hb = key
        self.nc = nc = build_program(with_score_bias=with_r,
                                     with_v_bias=with_hb)
        bass2jax.install_neuronx_cc_hook()

        partition_name = (
            nc.partition_id_tensor.name if nc.partition_id_tensor else None)
        in_names, out_names, out_avals, zero_outs = [], [], [], []
        for alloc in nc.m.functions[0].allocations:
            if not isinstance(alloc, mybir.MemoryLocationSet):
                continue
            name = alloc.memorylocations[0].name
            if alloc.kind == "ExternalInput":
                if name != partition_name:
                    in_names.append(name)
            elif alloc.kind == "ExternalOutput":
                shape = tuple(alloc.tensor_shape)
                dtype = mybir.dt.np(alloc.dtype)
                out_names.append(name)
                out_avals.append(jax.core.ShapedArray(shape, dtype))
                zero_outs.append(np.zeros((N_CORES * shape[0], *shape[1:]),
                                          dtype))
        self.in_names = in_names
        self.out_names = out_names
        n_ops = len(in_names) + len(out_names)

        def _body(*args):
            operands = list(args)
            if partition_name is not None:
                operands.append(bass2jax.partition_id_tensor())
            outs = bass2jax._bass_exec_p.bind(
                *operands,
                out_avals=tuple(out_avals),
                in_names=tuple(in_names + out_names +
                               ([partition_name] if partition_name else [])),
                out_names=tuple(out_names),
                lowering_input_output_aliases=(),
                sim_require_finite=True,
                sim_require_nnan=True,
                nc=nc,
            )
            return tuple(outs)

        devices = jax.devices()[:N_CORES]
        mesh = Mesh(np.asarray(devices), ("core",))
        self.sharding = NamedSharding(mesh, PartitionSpec("core"))
        self.fn = jax.jit(
            shard_map(_body, mesh=mesh,
                      in_specs=(PartitionSpec("core"),) * n_ops,
                      out_specs=(PartitionSpec("core"),) * len(out_names),
                      check_rep=False),
            keep_unused=True,
        )
        self.dev_zeros = [jax.device_put(z, self.sharding) for z in zero_outs]
        jax.block_until_ready(self.dev_zeros)
        # reusable host-side concat buffers, keyed by input name
        self.host_buf = {}

    def buf(self, name, shape, dtype):
        b = self.host_buf.get(name)
        if b is None or b.shape != shape or b.dtype != dtype:
            b = np.empty(shape, dtype)
            self.host_buf[name] = b
        return b

    def run(self, arrays):
        """arrays: dict name -> concat ndarray [N_CORES*rows, cols]."""
        import jax
        dev_in = [jax.device_put(arrays[n], self.sharding)
                  for n in self.in_names]
        outs = self.fn(*dev_in, *self.dev_zeros)
        return {n: np.asarray(o) for n, o in zip(self.out_names, outs)}


_EXEC_CACHE = {}


def _get_exec(key):
    if key not in _EXEC_CACHE:
        _EXEC_CACHE[key] = _Exec(key)
    return _EXEC_CACHE[key]


def prepare_inputs(ex, content, style, content_key, style_key, f_w, f_b,
                   g_w, g_b, h_w, h_b):
    """Fill ex's concat host buffers (fp16 wire format). Returns dict."""
    content = np.asarray(content)
    style = np.asarray(style)
    content_key = np.asarray(content_key)
    style_key = np.asarray(style_key)
    CC = C // 128
    C2 = C // 2
    GW = 2 * HW + 2 * C

    wT_1 = (np.asarray(g_w).T.astype(np.float32) @
            np.asarray(f_w).astype(np.float32)).astype(np.float16)
    hwT_1 = np.asarray(h_w).T.astype(np.float16)

    # host-side per-(b, channel) stats over all HW pixels (ddof=1 + EPS)
    cf = content.reshape(B, C, HW)
    mu_b = cf.mean(axis=2)                                   # [B, C]
    var_b = cf.var(axis=2, ddof=1) + 1e-5
    rstd_b = 1.0 / np.sqrt(var_b)
    musr = ex.buf("musr", (N_CORES * 128, 2 * CC), np.float32)
    for core in range(N_CORES):
        b = core // 2
        blk = musr[core * 128:(core + 1) * 128]
        blk[:, :CC] = mu_b[b].reshape(CC, 128).T
        blk[:, CC:] = rstd_b[b].reshape(CC, 128).T

    catq = ex.buf("catq", (N_CORES * C, 2 * Q), np.float16)
    gath = ex.buf("gath", (N_CORES * C2, GW), np.float16)

    def fill(core):
        b, h = divmod(core, 2)
        r = slice(core * C, (core + 1) * C)
        qs = slice(h * Q, (h + 1) * Q)
        cq = catq[r]
        cq[:, :Q] = content_key[b].reshape(C, HW)[:, qs]
        cq[:, Q:] = content[b].reshape(C, HW)[:, qs]
        hs = slice(h * C2, (h + 1) * C2)
        g = gath[core * C2:(core + 1) * C2]
        g[:, :HW] = style_key[b].reshape(C, HW)[hs]
        g[:, HW:2 * HW] = style[b].reshape(C, HW)[hs]
        g[:, 2 * HW:2 * HW + C] = wT_1[hs]
        g[:, 2 * HW + C:] = hwT_1[hs]

    with ThreadPoolExecutor(max_workers=8) as tp:
        list(tp.map(fill, range(N_CORES)))

    onesk = ex.buf("onesk", (N_CORES * 128, 1), np.float32)
    onesk[:] = 1.0
    m = {"catq": catq, "gath": gath, "musr": musr, "onesk": onesk}

    with_r = bool(np.any(f_b))
    with_hb = bool(np.any(h_b))
    if with_r or with_hb:
        onesr = ex.buf("onesr", (N_CORES * 1, 128), np.float16)
        onesr[:] = 1.0
        m["onesr"] = onesr
    if with_r:
        u = np.asarray(g_w).T.astype(np.float64) @ np.asarray(f_b, np.float64)
        rb = ex.buf("rbias", (N_CORES * 1, HW), np.float16)
        for core in range(N_CORES):
            b = core // 2
            rb[core] = (u @ style_key[b].reshape(C, HW).astype(np.float64))
        m["rbias"] = rb
    if with_hb:
        hb = ex.buf("hb", (N_CORES * 1, C), np.float16)
        hb[:] = np.asarray(h_b, np.float16)[None, :]
        m["hb"] = hb
    return m


def _variant_key(f_b, h_b):
    return (bool(np.any(f_b)), bool(np.any(h_b)))


def kernel(**inputs):
    key = _variant_key(inputs["f_b"], inputs["h_b"])
    ex = _get_exec(key)
    arrays = prepare_inputs(ex, **inputs)
    res = ex.run(arrays)
    o = res["out"]                               # [8*C, Q] fp16
    out = np.empty((B, C, HW), np.float32)
    for core in range(N_CORES):
        b, h = divmod(core, 2)
        out[b][:, h * Q:(h + 1) * Q] = o[core * C:(core + 1) * C]
    return out.reshape(B, C, H, W)


if __name__ == "__main__":
    rng = np.random.default_rng(0)
    inputs = {
        "content": rng.standard_normal((B, C, H, W)).astype(np.float32),
        "style": rng.standard_normal((B, C, H, W)).astype(np.float32),
        "content_key": rng.standard_normal((B, C, H, W)).astype(np.float32),
        "style_key": rng.standard_normal((B, C, H, W)).astype(np.float32),
        "f_w": (rng.standard_normal((C, C)) * 0.02).astype(np.float32),
        "f_b": np.zeros(C, np.float32),
        "g_w": (rng.standard_normal((C, C)) * 0.02).astype(np.float32),
        "g_b": np.zeros(C, np.float32),
        "h_w": (rng.standard_normal((C, C)) * 0.02).astype(np.float32),
        "h_b": np.zeros(C, np.float32),
    }
    t0 = time.time()
    out = kernel(**inputs)
    print("kernel done", out.shape, out.dtype, time.time() - t0)
    t0 = time.time()
    out = kernel(**inputs)
    print("second call", time.time() - t0)
